# revision 20
# baseline (speedup 1.0000x reference)
"""BitLinear-1.58 Trainium2 kernel (8-core SPMD), v2.

out = (clip(round(x * s), -128, 127) @ w.T) / s / weight_scale + bias,
s = 127 / clip(rowmax|x|, 1e-5),  w in {0,1} (int32), x [4096, 8192] f32.

Sharding: token dim split 4 ways x out-feature dim split 2 ways -> 8 cores.
Each core: x-block [1024, 8192], weight-block [4096, 8192], out-block
[1024, 4096].

v3 strategy (v2's 454us pre-matmul stall was the double-pass x pipeline +
XBAR-transpose serialization):
  w:  staged on HOST as bf16, pre-transposed into matmul-ready slab blocks
      [nt*8+kq, 128k, 8kol*512n] -> each (nt,kq) slab is ONE contiguous 1 MB
      DMA. No on-chip convert, no XBAR transpose, no gpsimd.
  x:  staged BOTH natural (rowmax pass) and pre-transposed x_T (quantize).
      Phase A: rowmax|x| (DVE) -> s = exact 127/m; s flipped to a free-dim
      row via a tiny DRAM round-trip, broadcast to s_bc [128,1024].
      Quantize: per 128-k slab of x_T, DVE mult by s_bc then DVE 2-op
      (+MAGIC,-MAGIC) RNE round -> bf16 written straight into resident
      xq [128k, 64ko, 1024t] (16 MB). No XBAR; xq is produced ko-major so
      matmuls can start as soon as rowmax + the first k-slabs are done.
  mm: 8 PSUM banks accumulate [128t, 512n] over all 64 ko; ACT drains with
      per-token scale d=1/(s*ws); DVE adds bias; store [t, n] natural.

Exactness: fl(fl(x*s)+MAGIC)-MAGIC = RNE-round-to-int of fl(x*s), matching
jnp.round bit-exactly (|x*s| <= 127 < 2^23 so the magic trick is exact; ints
<= 127 are exact in bf16). Every partial sum < 2^24 so fp32 PSUM accumulation
is exact. clip never binds since |x*s| <= 127(1+2eps) < 127.5.
"""
import os as _os
import sys

sys.path.insert(0, "/opt/trn_rl_repo")

from contextlib import ExitStack

import ml_dtypes
import numpy as np

import concourse.bass as bass
import concourse.tile as tile
from concourse import bacc, mybir
from concourse.bass import ts
from concourse.bass_utils import run_bass_kernel_spmd

TOKENS, IN_F, OUT_F = 4096, 8192, 8192
A_SPLIT, B_SPLIT = 4, 2      # token blocks x outfeature blocks = 8 cores
T_LOC = TOKENS // A_SPLIT    # 1024
N_LOC = OUT_F // B_SPLIT     # 4096
P = 128
KO = IN_F // P               # 64 k-tiles of 128
TT = T_LOC // P              # 8 token tiles
NT = N_LOC // 512            # 8 n-tiles of 512
KQ = 16                      # k chunks for weight slabs
KO_Q = KO // KQ              # 4
XQRT = 4                     # x processed in [128, 2048] quarters
QW = IN_F // XQRT            # 2048
MAGIC = float(np.float32(1.5 * 2 ** 23))

_NT_DBG = int(_os.environ.get("BITLIN_NT", NT))
_CACHE = {}


def _exact_div127(nc, dst, m, pool, pfx):
    """dst = correctly-rounded IEEE 127/m (f32).

    nc.vector.reciprocal is correctly rounded (verified bit-exact on HW), so
    q0 = fl(127*r0) is within ~1 ulp of 127/m; one Markstein step with an
    exact Dekker residual lands on the correctly-rounded quotient."""
    f32 = mybir.dt.float32
    A = mybir.AluOpType
    sh = list(m.shape)
    t = {k: pool.tile(sh, f32, name=f"{pfx}_{k}", tag=f"dv_{k}")
         for k in ("r0", "q0", "tmp", "hh", "ll", "mh", "ml", "p", "err", "e")}
    nc.vector.reciprocal(t["r0"][:], m[:])
    nc.vector.tensor_scalar_mul(t["q0"][:], t["r0"][:], 127.0)
    C = float(2 ** 12 + 1)
    nc.vector.tensor_scalar_mul(t["tmp"][:], t["q0"][:], C)
    nc.vector.tensor_tensor(t["hh"][:], t["tmp"][:], t["q0"][:], A.subtract)
    nc.vector.tensor_tensor(t["hh"][:], t["tmp"][:], t["hh"][:], A.subtract)
    nc.vector.tensor_tensor(t["ll"][:], t["q0"][:], t["hh"][:], A.subtract)
    nc.vector.tensor_scalar_mul(t["tmp"][:], m[:], C)
    nc.vector.tensor_tensor(t["mh"][:], t["tmp"][:], m[:], A.subtract)
    nc.vector.tensor_tensor(t["mh"][:], t["tmp"][:], t["mh"][:], A.subtract)
    nc.vector.tensor_tensor(t["ml"][:], m[:], t["mh"][:], A.subtract)
    nc.vector.tensor_tensor(t["p"][:], t["q0"][:], m[:], A.mult)
    nc.vector.tensor_tensor(t["err"][:], t["hh"][:], t["mh"][:], A.mult)
    nc.vector.tensor_tensor(t["err"][:], t["err"][:], t["p"][:], A.subtract)
    nc.vector.tensor_tensor(t["tmp"][:], t["hh"][:], t["ml"][:], A.mult)
    nc.vector.tensor_tensor(t["err"][:], t["err"][:], t["tmp"][:], A.add)
    nc.vector.tensor_tensor(t["tmp"][:], t["ll"][:], t["mh"][:], A.mult)
    nc.vector.tensor_tensor(t["err"][:], t["err"][:], t["tmp"][:], A.add)
    nc.vector.tensor_tensor(t["tmp"][:], t["ll"][:], t["ml"][:], A.mult)
    nc.vector.tensor_tensor(t["err"][:], t["err"][:], t["tmp"][:], A.add)
    nc.vector.tensor_scalar(t["e"][:], t["p"][:], 127.0, -1.0, A.subtract, A.mult)
    nc.vector.tensor_tensor(t["e"][:], t["e"][:], t["err"][:], A.subtract)
    nc.vector.tensor_tensor(t["tmp"][:], t["e"][:], t["r0"][:], A.mult)
    nc.vector.tensor_tensor(dst[:], t["q0"][:], t["tmp"][:], A.add)


def _build():
    if "nc" in _CACHE:
        return _CACHE["nc"]

    nc = bacc.Bacc("TRN2", target_bir_lowering=False, debug=False, num_devices=8)
    f32, bf16 = mybir.dt.float32, mybir.dt.bfloat16
    A = mybir.AluOpType
    Copy = mybir.ActivationFunctionType.Copy

    xb = nc.dram_tensor("xb", [T_LOC, IN_F], f32, kind="ExternalInput").ap()
    xt = nc.dram_tensor("xt", [IN_F, T_LOC], f32, kind="ExternalInput").ap()
    eye = nc.dram_tensor("eye", [P, P], f32, kind="ExternalInput").ap()
    # weight staged on host: [nt*KQ+kq, 128 k-part, kol*512+n] bf16, each row
    # of dim0 is one contiguous matmul-ready slab
    wt = nc.dram_tensor("wt", [NT * KQ, P, KO_Q * 512], bf16,
                        kind="ExternalInput").ap()
    bb = nc.dram_tensor("bb", [N_LOC], f32, kind="ExternalInput").ap()
    ws = nc.dram_tensor("ws", [1], f32, kind="ExternalInput").ap()
    ob = nc.dram_tensor("ob", [T_LOC, N_LOC], f32, kind="ExternalOutput").ap()

    with tile.TileContext(nc) as tc:
        with ExitStack() as ctx:
            small = ctx.enter_context(tc.tile_pool(name="small", bufs=1))
            xqp = ctx.enter_context(tc.tile_pool(name="xq", bufs=1))
            xq = xqp.tile([P, KO, T_LOC], bf16)   # 128 KB/partition, resident

            # weight-scale reciprocal (per-partition [P,1] broadcast)
            ws_sb = small.tile([1, 1], f32)
            nc.sync.dma_start(ws_sb[:], ws[None, :])
            rws = small.tile([1, 1], f32)
            nc.vector.reciprocal(rws[:], ws_sb[:])
            rws_b = small.tile([P, 1], f32)
            nc.gpsimd.partition_broadcast(rws_b[:], rws[:])

            d_all = small.tile([P, TT], f32)      # per-token out scale 1/s/ws
            m_all = small.tile([P, TT], f32)
            s_all = small.tile([P, TT], f32)

            # ---- Phase A machinery: rowmax -> s per token-half ----
            s_lin = small.tile([1, T_LOC], f32)
            s_bc = small.tile([P, T_LOC], f32)
            eye_sb = small.tile([P, P], f32)
            nc.sync.dma_start(eye_sb[:], eye[:, :])
            pha = ctx.enter_context(tc.tile_pool(name="phA", bufs=3))
            TPT = TT // 2  # t-tiles per token-half

            def phase_a(th, load_eng):
                """rowmax|x| for t-tiles of half th, batched exact s = 127/m,
                d = 1/(s*ws), and flip s into s_bc[:, th-slice] via a PE
                transpose (partition-major -> free-dim row)."""
                sl = slice(th * TPT, (th + 1) * TPT)
                for tt in range(th * TPT, (th + 1) * TPT):
                    m4 = small.tile([P, XQRT], f32, tag="m4", name=f"m4_{tt}")
                    for q in range(XQRT):
                        xh = pha.tile([P, QW], f32, tag="xa")
                        eng = load_eng[q % len(load_eng)]
                        eng.dma_start(xh[:], xb[ts(tt, P), ts(q, QW)])
                        nc.vector.tensor_reduce(
                            m4[:, q : q + 1], xh[:], mybir.AxisListType.X,
                            A.max, apply_absolute_value=True)
                    nc.vector.tensor_reduce(m_all[:, tt : tt + 1], m4[:],
                                            mybir.AxisListType.X, A.max)
                # batched exact-division pass for the half's 4 t-tiles
                nc.vector.tensor_scalar_max(m_all[:, sl], m_all[:, sl], 1e-5)
                _exact_div127(nc, s_all[:, sl], m_all[:, sl], small, f"dv{th}")
                nc.vector.reciprocal(d_all[:, sl], s_all[:, sl])
                nc.vector.tensor_scalar(d_all[:, sl], d_all[:, sl],
                                        rws_b[:, 0:1], None, A.mult)
                # flip s to a free-dim row: PE transpose into a borrowed PSUM
                # slice, copy to SBUF, then 512B per-row hops to partition 0
                tr = pp.tile([P, 512], f32, tag="acc", name=f"str_{th}")
                nc.tensor.transpose(tr[0:TPT, 0:P], s_all[:, sl], eye_sb[:])
                s8 = small.tile([TPT, P], f32, tag="s8", name=f"s8_{th}")
                nc.vector.tensor_copy(s8[:], tr[0:TPT, 0:P])
                for i, tt in enumerate(range(th * TPT, (th + 1) * TPT)):
                    eng = nc.sync if tt % 2 == 0 else nc.scalar
                    eng.dma_start(s_lin[0:1, ts(tt, P)], s8[i : i + 1, :])
                nc.gpsimd.partition_broadcast(s_bc[:, ts(th, 512)],
                                              s_lin[0:1, ts(th, 512)])

            # ---- Phase C: weight slabs + GEMM in token-half passes.
            # PSUM generations are 4 banks (4 t-tiles x one 512-n chunk), so
            # consecutive generations overlap in the 8 banks (no drain
            # stalls). Emission order is arranged around the in-order DVE
            # queue: phaseA(th0) -> fused startup (quantize th0 paced with
            # nt0+nt1 matmuls of token-half 0) -> phaseA(th1) + quantize th1
            # (hidden under tp0's remaining matmuls) -> the rest. ----
            phb = ctx.enter_context(tc.tile_pool(name="phB", bufs=6))
            slp = ctx.enter_context(tc.tile_pool(name="slab", bufs=6))
            pp = ctx.enter_context(tc.tile_pool(name="psum", bufs=8, space="PSUM"))
            op = ctx.enter_context(tc.tile_pool(name="outp", bufs=2))
            bip = ctx.enter_context(tc.tile_pool(name="bias", bufs=1))

            def bias_tile(nt):
                b_row = bip.tile([1, 512], f32, tag="brow")
                nc.scalar.dma_start(b_row[:], bb[None, ts(nt, 512)])
                b_bc = bip.tile([P, 512], f32, tag="bbc")
                nc.gpsimd.partition_broadcast(b_bc[:], b_row[:])
                return b_bc

            def quantize(ko, th):
                # fl(x*s) exact (matches reference), then RNE round-to-int
                # via two-op magic -> bf16 (ints <= 127 exact). th0 loads
                # ride the pre-MM-idle ACT ring; th1 loads go via gpsimd
                # SWDGE so drains/stores on ACT stay responsive.
                xsl = phb.tile([P, 512], f32, tag="xsl")
                eng = nc.scalar if th == 0 else nc.gpsimd
                eng.dma_start(xsl[:], xt[ts(ko, P), ts(th, 512)])
                nc.vector.tensor_tensor(xsl[:], xsl[:],
                                        s_bc[:, ts(th, 512)], A.mult)
                nc.vector.tensor_scalar(xq[:, ko, ts(th, 512)], xsl[:],
                                        MAGIC, -MAGIC, A.add, A.add)

            def mm_group(psums, tp, kq, slab):
                for kol in range(KO_Q):
                    ko = kq * KO_Q + kol
                    for ti in range(TPT):
                        t = tp * TPT + ti
                        nc.tensor.matmul(
                            psums[ti][:], xq[:, ko, ts(t, P)],
                            slab[:, ts(kol, 512)],
                            start=(ko == 0), stop=(ko == KO - 1))

            def drains(psums, tp, nt, b_bc):
                for ti in range(TPT):
                    t = tp * TPT + ti
                    o_sb = op.tile([P, 512], f32, tag="osb")
                    nc.scalar.activation(o_sb[:], psums[ti][:], Copy,
                                         scale=d_all[:, t : t + 1])
                    nc.vector.tensor_tensor(o_sb[:], o_sb[:], b_bc[:], A.add)
                    nc.scalar.dma_start(ob[ts(t, P), ts(nt, 512)], o_sb[:])

            def new_psums(tp, nt):
                return [pp.tile([P, 512], f32, tag="acc",
                                name=f"ps_{tp}_{nt}_{ti}")
                        for ti in range(TPT)]

            def load_slab(nt, kq):
                slab = slp.tile([P, KO_Q * 512], bf16, tag="slab")
                nc.sync.dma_start(slab[:], wt[nt * KQ + kq])
                return slab

            phase_a(0, [nc.sync, nc.scalar])
            # fused startup: token-half 0, nt 0+1, quantize th0 paced per kq
            b0, b1 = bias_tile(0), bias_tile(1)
            ps0, ps1 = new_psums(0, 0), new_psums(0, 1)
            for kq in range(KQ):
                for kol in range(KO_Q):
                    quantize(kq * KO_Q + kol, 0)
                slab0 = load_slab(0, kq)
                slab1 = load_slab(1, kq)
                mm_group(ps0, 0, kq, slab0)
                mm_group(ps1, 0, kq, slab1)
            drains(ps0, 0, 0, b0)
            drains(ps1, 0, 1, b1)

            # token-half 1 prep runs on DVE while tp0's matmuls continue;
            # its x loads go through the idle gpsimd SWDGE ring so they don't
            # contend with the latency-critical xsl loads on the ACT ring
            phase_a(1, [nc.gpsimd])
            for ko in range(KO):
                quantize(ko, 1)

            for tp in range(2):
                for nt in range(NT):
                    if tp == 0 and nt < 2:
                        continue  # covered by the fused startup segment
                    b_bc = bias_tile(nt)
                    psums = new_psums(tp, nt)
                    for kq in range(KQ):
                        slab = load_slab(nt, kq)
                        mm_group(psums, tp, kq, slab)
                    drains(psums, tp, nt, b_bc)

    nc.compile()
    _CACHE["nc"] = nc
    return nc


def _stage_weight_core(weight, j):
    """Host-side: core j's weight rows -> matmul-ready bf16 slab blocks.

    Returns [NT*KQ, P, KO_Q*512] bf16 where block b = nt*KQ+kq holds
    w[nt*512+n, kq*(P*KO_Q) + kol*P + p] at [b, p, kol*512+n]."""
    wc = weight[j * N_LOC:(j + 1) * N_LOC]              # [4096 n, 8192 k]
    v = wc.astype(ml_dtypes.bfloat16).T                 # [8192 k, 4096 n]
    v = v.reshape(KQ, KO_Q, P, NT, 512)                 # k=(kq,kol,p) n=(nt,j)
    v = v.transpose(3, 0, 2, 1, 4)                      # [nt, kq, p, kol, j]
    return np.ascontiguousarray(v).reshape(NT * KQ, P, KO_Q * 512)


def _stage_inputs(x, weight, weight_scale, bias):
    x = np.ascontiguousarray(np.asarray(x, dtype=np.float32))
    weight = np.asarray(weight, dtype=np.int32)
    weight_scale = np.asarray(weight_scale, dtype=np.float32).reshape(1)
    bias = np.ascontiguousarray(np.asarray(bias, dtype=np.float32))

    wt_by_j = [_stage_weight_core(weight, j) for j in range(B_SPLIT)]
    xt_by_i = [np.ascontiguousarray(x[i * T_LOC:(i + 1) * T_LOC].T)
               for i in range(A_SPLIT)]
    eye = np.eye(P, dtype=np.float32)
    in_maps = []
    for c in range(8):
        i, j = c // B_SPLIT, c % B_SPLIT
        in_maps.append({
            "xb": x[i * T_LOC:(i + 1) * T_LOC],
            "xt": xt_by_i[i],
            "wt": wt_by_j[j],
            "bb": bias[j * N_LOC:(j + 1) * N_LOC],
            "ws": weight_scale,
            "eye": eye,
        })
    return in_maps


def kernel(x, weight, weight_scale, bias):
    nc = _build()
    in_maps = _stage_inputs(x, weight, weight_scale, bias)
    res = run_bass_kernel_spmd(nc, in_maps, list(range(8))).results

    out = np.empty((TOKENS, OUT_F), dtype=np.float32)
    for c in range(8):
        i, j = c // B_SPLIT, c % B_SPLIT
        out[i * T_LOC:(i + 1) * T_LOC, j * N_LOC:(j + 1) * N_LOC] = res[c]["ob"]
    return out


# revision 22
# speedup vs baseline: 1.1673x; 1.1673x over previous
"""BitLinear-1.58 Trainium2 kernel (8-core SPMD), v2.

out = (clip(round(x * s), -128, 127) @ w.T) / s / weight_scale + bias,
s = 127 / clip(rowmax|x|, 1e-5),  w in {0,1} (int32), x [4096, 8192] f32.

Sharding: token dim split 4 ways x out-feature dim split 2 ways -> 8 cores.
Each core: x-block [1024, 8192], weight-block [4096, 8192], out-block
[1024, 4096].

Final design (2.645 ms baseline -> ~1.05 ms; PE floor is 874 us):
  w:  staged on HOST as bf16, pre-transposed into matmul-ready slab blocks
      [nt*KQ+kq, 128k, KO_Q*512n] -> each (nt,kq) slab is ONE contiguous
      512KB DMA. No on-chip convert, no XBAR transpose.
  x:  staged BOTH natural (rowmax pass) and pre-transposed x_T (quantize).
      Phase A (per token-half): rowmax|x| (DVE, dual-ring loads) -> batched
      exact s = 127/m -> s flipped partition->free via PE transpose (eye
      input) + per-row 512B hops -> broadcast s_bc. Quantize per x_T k-slab:
      DVE mult by s_bc then DVE 2-op (+MAGIC,-MAGIC) RNE round -> bf16
      straight into resident xq [128k, 64ko, 1024t] (16 MB), ko-major.
  mm: token-half passes; PSUM generations of 4 banks (4 t-tiles x 512n) so
      consecutive generations overlap in the 8 banks; a fused startup
      segment runs nt0+nt1 of half 0 kq-synchronized with the quantize
      producer. ACT drains with per-token scale d=1/(s*ws); DVE adds bias;
      stores [t, n] natural. Weight streamed twice (once per token-half).
      Engine-queue placement is deliberate: slabs on SP ring, th0
      xsl/drains/stores on ACT ring, th1 x+xsl loads on gpsimd SWDGE, th1
      prep emitted mid-loop so its PE-transpose never stalls the PE queue.

Exactness: fl(fl(x*s)+MAGIC)-MAGIC = RNE-round-to-int of fl(x*s), matching
jnp.round bit-exactly (|x*s| <= 127 < 2^23 so the magic trick is exact; ints
<= 127 are exact in bf16). Every partial sum < 2^24 so fp32 PSUM accumulation
is exact. clip never binds since |x*s| <= 127(1+2eps) < 127.5.
"""
import os as _os
import sys

sys.path.insert(0, "/opt/trn_rl_repo")

from contextlib import ExitStack

import ml_dtypes
import numpy as np

import concourse.bass as bass
import concourse.tile as tile
from concourse import bacc, mybir
from concourse.bass import ts
from concourse.bass_utils import run_bass_kernel_spmd

TOKENS, IN_F, OUT_F = 4096, 8192, 8192
A_SPLIT, B_SPLIT = 4, 2      # token blocks x outfeature blocks = 8 cores
T_LOC = TOKENS // A_SPLIT    # 1024
N_LOC = OUT_F // B_SPLIT     # 4096
P = 128
KO = IN_F // P               # 64 k-tiles of 128
TT = T_LOC // P              # 8 token tiles
NT = N_LOC // 512            # 8 n-tiles of 512
KQ = 16                      # k chunks for weight slabs
KO_Q = KO // KQ              # 4
XQRT = 4                     # x processed in [128, 2048] quarters
QW = IN_F // XQRT            # 2048
MAGIC = float(np.float32(1.5 * 2 ** 23))

_NT_DBG = int(_os.environ.get("BITLIN_NT", NT))
_CACHE = {}


def _exact_div127(nc, dst, m, pool, pfx):
    """dst = correctly-rounded IEEE 127/m (f32).

    nc.vector.reciprocal is correctly rounded (verified bit-exact on HW), so
    q0 = fl(127*r0) is within ~1 ulp of 127/m; one Markstein step with an
    exact Dekker residual lands on the correctly-rounded quotient."""
    f32 = mybir.dt.float32
    A = mybir.AluOpType
    sh = list(m.shape)
    t = {k: pool.tile(sh, f32, name=f"{pfx}_{k}", tag=f"dv_{k}")
         for k in ("r0", "q0", "tmp", "hh", "ll", "mh", "ml", "p", "err", "e")}
    nc.vector.reciprocal(t["r0"][:], m[:])
    nc.vector.tensor_scalar_mul(t["q0"][:], t["r0"][:], 127.0)
    C = float(2 ** 12 + 1)
    nc.vector.tensor_scalar_mul(t["tmp"][:], t["q0"][:], C)
    nc.vector.tensor_tensor(t["hh"][:], t["tmp"][:], t["q0"][:], A.subtract)
    nc.vector.tensor_tensor(t["hh"][:], t["tmp"][:], t["hh"][:], A.subtract)
    nc.vector.tensor_tensor(t["ll"][:], t["q0"][:], t["hh"][:], A.subtract)
    nc.vector.tensor_scalar_mul(t["tmp"][:], m[:], C)
    nc.vector.tensor_tensor(t["mh"][:], t["tmp"][:], m[:], A.subtract)
    nc.vector.tensor_tensor(t["mh"][:], t["tmp"][:], t["mh"][:], A.subtract)
    nc.vector.tensor_tensor(t["ml"][:], m[:], t["mh"][:], A.subtract)
    nc.vector.tensor_tensor(t["p"][:], t["q0"][:], m[:], A.mult)
    nc.vector.tensor_tensor(t["err"][:], t["hh"][:], t["mh"][:], A.mult)
    nc.vector.tensor_tensor(t["err"][:], t["err"][:], t["p"][:], A.subtract)
    nc.vector.tensor_tensor(t["tmp"][:], t["hh"][:], t["ml"][:], A.mult)
    nc.vector.tensor_tensor(t["err"][:], t["err"][:], t["tmp"][:], A.add)
    nc.vector.tensor_tensor(t["tmp"][:], t["ll"][:], t["mh"][:], A.mult)
    nc.vector.tensor_tensor(t["err"][:], t["err"][:], t["tmp"][:], A.add)
    nc.vector.tensor_tensor(t["tmp"][:], t["ll"][:], t["ml"][:], A.mult)
    nc.vector.tensor_tensor(t["err"][:], t["err"][:], t["tmp"][:], A.add)
    nc.vector.tensor_scalar(t["e"][:], t["p"][:], 127.0, -1.0, A.subtract, A.mult)
    nc.vector.tensor_tensor(t["e"][:], t["e"][:], t["err"][:], A.subtract)
    nc.vector.tensor_tensor(t["tmp"][:], t["e"][:], t["r0"][:], A.mult)
    nc.vector.tensor_tensor(dst[:], t["q0"][:], t["tmp"][:], A.add)


def _build():
    if "nc" in _CACHE:
        return _CACHE["nc"]

    nc = bacc.Bacc("TRN2", target_bir_lowering=False, debug=False, num_devices=8)
    f32, bf16 = mybir.dt.float32, mybir.dt.bfloat16
    A = mybir.AluOpType
    Copy = mybir.ActivationFunctionType.Copy

    xb = nc.dram_tensor("xb", [T_LOC, IN_F], f32, kind="ExternalInput").ap()
    xt = nc.dram_tensor("xt", [IN_F, T_LOC], f32, kind="ExternalInput").ap()
    eye = nc.dram_tensor("eye", [P, P], f32, kind="ExternalInput").ap()
    # weight staged on host: [nt*KQ+kq, 128 k-part, kol*512+n] bf16, each row
    # of dim0 is one contiguous matmul-ready slab
    wt = nc.dram_tensor("wt", [NT * KQ, P, KO_Q * 512], bf16,
                        kind="ExternalInput").ap()
    bb = nc.dram_tensor("bb", [N_LOC], f32, kind="ExternalInput").ap()
    ws = nc.dram_tensor("ws", [1], f32, kind="ExternalInput").ap()
    ob = nc.dram_tensor("ob", [T_LOC, N_LOC], f32, kind="ExternalOutput").ap()

    with tile.TileContext(nc) as tc:
        with ExitStack() as ctx:
            small = ctx.enter_context(tc.tile_pool(name="small", bufs=1))
            xqp = ctx.enter_context(tc.tile_pool(name="xq", bufs=1))
            xq = xqp.tile([P, KO, T_LOC], bf16)   # 128 KB/partition, resident

            # weight-scale reciprocal (per-partition [P,1] broadcast)
            ws_sb = small.tile([1, 1], f32)
            nc.sync.dma_start(ws_sb[:], ws[None, :])
            rws = small.tile([1, 1], f32)
            nc.vector.reciprocal(rws[:], ws_sb[:])
            rws_b = small.tile([P, 1], f32)
            nc.gpsimd.partition_broadcast(rws_b[:], rws[:])

            d_all = small.tile([P, TT], f32)      # per-token out scale 1/s/ws
            m_all = small.tile([P, TT], f32)
            s_all = small.tile([P, TT], f32)

            # ---- Phase A machinery: rowmax -> s per token-half ----
            s_lin = small.tile([1, T_LOC], f32)
            s_bc = small.tile([P, T_LOC], f32)
            eye_sb = small.tile([P, P], f32)
            nc.sync.dma_start(eye_sb[:], eye[:, :])
            pha = ctx.enter_context(tc.tile_pool(name="phA", bufs=3))
            TPT = TT // 2  # t-tiles per token-half

            def phase_a(th, load_eng):
                """rowmax|x| for t-tiles of half th, batched exact s = 127/m,
                d = 1/(s*ws), and flip s into s_bc[:, th-slice] via a PE
                transpose (partition-major -> free-dim row)."""
                sl = slice(th * TPT, (th + 1) * TPT)
                for tt in range(th * TPT, (th + 1) * TPT):
                    m4 = small.tile([P, XQRT], f32, tag="m4", name=f"m4_{tt}")
                    for q in range(XQRT):
                        xh = pha.tile([P, QW], f32, tag="xa")
                        eng = load_eng[q % len(load_eng)]
                        eng.dma_start(xh[:], xb[ts(tt, P), ts(q, QW)])
                        nc.vector.tensor_reduce(
                            m4[:, q : q + 1], xh[:], mybir.AxisListType.X,
                            A.max, apply_absolute_value=True)
                    nc.vector.tensor_reduce(m_all[:, tt : tt + 1], m4[:],
                                            mybir.AxisListType.X, A.max)
                # batched exact-division pass for the half's 4 t-tiles
                nc.vector.tensor_scalar_max(m_all[:, sl], m_all[:, sl], 1e-5)
                _exact_div127(nc, s_all[:, sl], m_all[:, sl], small, f"dv{th}")
                nc.vector.reciprocal(d_all[:, sl], s_all[:, sl])
                nc.vector.tensor_scalar(d_all[:, sl], d_all[:, sl],
                                        rws_b[:, 0:1], None, A.mult)
                # flip s to a free-dim row: PE transpose into a borrowed PSUM
                # slice, copy to SBUF, then 512B per-row hops to partition 0
                tr = pp.tile([P, 512], f32, tag="acc", name=f"str_{th}")
                nc.tensor.transpose(tr[0:TPT, 0:P], s_all[:, sl], eye_sb[:])
                s8 = small.tile([TPT, P], f32, tag="s8", name=f"s8_{th}")
                nc.vector.tensor_copy(s8[:], tr[0:TPT, 0:P])
                for i, tt in enumerate(range(th * TPT, (th + 1) * TPT)):
                    eng = nc.sync if tt % 2 == 0 else nc.scalar
                    eng.dma_start(s_lin[0:1, ts(tt, P)], s8[i : i + 1, :])
                nc.gpsimd.partition_broadcast(s_bc[:, ts(th, 512)],
                                              s_lin[0:1, ts(th, 512)])

            # ---- Phase C: weight slabs + GEMM in token-half passes.
            # PSUM generations are 4 banks (4 t-tiles x one 512-n chunk), so
            # consecutive generations overlap in the 8 banks (no drain
            # stalls). Emission order is arranged around the in-order DVE
            # queue: phaseA(th0) -> fused startup (quantize th0 paced with
            # nt0+nt1 matmuls of token-half 0) -> phaseA(th1) + quantize th1
            # (hidden under tp0's remaining matmuls) -> the rest. ----
            phb = ctx.enter_context(tc.tile_pool(name="phB", bufs=6))
            slp = ctx.enter_context(tc.tile_pool(name="slab", bufs=6))
            pp = ctx.enter_context(tc.tile_pool(name="psum", bufs=8, space="PSUM"))
            op = ctx.enter_context(tc.tile_pool(name="outp", bufs=2))
            bip = ctx.enter_context(tc.tile_pool(name="bias", bufs=1))

            def bias_tile(nt):
                b_row = bip.tile([1, 512], f32, tag="brow")
                nc.scalar.dma_start(b_row[:], bb[None, ts(nt, 512)])
                b_bc = bip.tile([P, 512], f32, tag="bbc")
                nc.gpsimd.partition_broadcast(b_bc[:], b_row[:])
                return b_bc

            def quantize(ko, th):
                # fl(x*s) exact (matches reference), then RNE round-to-int
                # via two-op magic -> bf16 (ints <= 127 exact). th0 loads
                # ride the pre-MM-idle ACT ring; th1 loads go via gpsimd
                # SWDGE so drains/stores on ACT stay responsive.
                xsl = phb.tile([P, 512], f32, tag="xsl")
                eng = nc.scalar if th == 0 else nc.gpsimd
                eng.dma_start(xsl[:], xt[ts(ko, P), ts(th, 512)])
                nc.vector.tensor_tensor(xsl[:], xsl[:],
                                        s_bc[:, ts(th, 512)], A.mult)
                nc.vector.tensor_scalar(xq[:, ko, ts(th, 512)], xsl[:],
                                        MAGIC, -MAGIC, A.add, A.add)

            def mm_group(psums, tp, kq, slab):
                for kol in range(KO_Q):
                    ko = kq * KO_Q + kol
                    for ti in range(TPT):
                        t = tp * TPT + ti
                        nc.tensor.matmul(
                            psums[ti][:], xq[:, ko, ts(t, P)],
                            slab[:, ts(kol, 512)],
                            start=(ko == 0), stop=(ko == KO - 1))

            def drains(psums, tp, nt, b_bc):
                for ti in range(TPT):
                    t = tp * TPT + ti
                    o_sb = op.tile([P, 512], f32, tag="osb")
                    nc.scalar.activation(o_sb[:], psums[ti][:], Copy,
                                         scale=d_all[:, t : t + 1])
                    nc.vector.tensor_tensor(o_sb[:], o_sb[:], b_bc[:], A.add)
                    nc.scalar.dma_start(ob[ts(t, P), ts(nt, 512)], o_sb[:])

            def new_psums(tp, nt):
                return [pp.tile([P, 512], f32, tag="acc",
                                name=f"ps_{tp}_{nt}_{ti}")
                        for ti in range(TPT)]

            def load_slab(nt, kq):
                slab = slp.tile([P, KO_Q * 512], bf16, tag="slab")
                nc.sync.dma_start(slab[:], wt[nt * KQ + kq])
                return slab

            phase_a(0, [nc.sync, nc.scalar])
            # fused startup: token-half 0, nt 0+1, quantize th0 paced per kq
            b0, b1 = bias_tile(0), bias_tile(1)
            ps0, ps1 = new_psums(0, 0), new_psums(0, 1)
            for kq in range(KQ):
                for kol in range(KO_Q):
                    quantize(kq * KO_Q + kol, 0)
                slab0 = load_slab(0, kq)
                slab1 = load_slab(1, kq)
                mm_group(ps0, 0, kq, slab0)
                mm_group(ps1, 0, kq, slab1)
            drains(ps0, 0, 0, b0)
            drains(ps1, 0, 1, b1)

            for tp in range(2):
                for nt in range(NT):
                    if tp == 0 and nt < 2:
                        continue  # covered by the fused startup segment
                    b_bc = bias_tile(nt)
                    psums = new_psums(tp, nt)
                    for kq in range(KQ):
                        slab = load_slab(nt, kq)
                        mm_group(psums, tp, kq, slab)
                    drains(psums, tp, nt, b_bc)
                    if tp == 0 and nt == 3:
                        # token-half 1 prep: emitted here so its PE-transpose
                        # sits in the PE queue where its DVE inputs are
                        # already done (no PE stall); x loads ride the idle
                        # gpsimd SWDGE ring
                        phase_a(1, [nc.gpsimd])
                        for ko in range(KO):
                            quantize(ko, 1)

    nc.compile()
    _CACHE["nc"] = nc
    return nc


def _stage_weight_core(weight, j):
    """Host-side: core j's weight rows -> matmul-ready bf16 slab blocks.

    Returns [NT*KQ, P, KO_Q*512] bf16 where block b = nt*KQ+kq holds
    w[nt*512+n, kq*(P*KO_Q) + kol*P + p] at [b, p, kol*512+n]."""
    wc = weight[j * N_LOC:(j + 1) * N_LOC]              # [4096 n, 8192 k]
    v = wc.astype(ml_dtypes.bfloat16).T                 # [8192 k, 4096 n]
    v = v.reshape(KQ, KO_Q, P, NT, 512)                 # k=(kq,kol,p) n=(nt,j)
    v = v.transpose(3, 0, 2, 1, 4)                      # [nt, kq, p, kol, j]
    return np.ascontiguousarray(v).reshape(NT * KQ, P, KO_Q * 512)


def _stage_inputs(x, weight, weight_scale, bias):
    x = np.ascontiguousarray(np.asarray(x, dtype=np.float32))
    weight = np.asarray(weight, dtype=np.int32)
    weight_scale = np.asarray(weight_scale, dtype=np.float32).reshape(1)
    bias = np.ascontiguousarray(np.asarray(bias, dtype=np.float32))

    wt_by_j = [_stage_weight_core(weight, j) for j in range(B_SPLIT)]
    xt_by_i = [np.ascontiguousarray(x[i * T_LOC:(i + 1) * T_LOC].T)
               for i in range(A_SPLIT)]
    eye = np.eye(P, dtype=np.float32)
    in_maps = []
    for c in range(8):
        i, j = c // B_SPLIT, c % B_SPLIT
        in_maps.append({
            "xb": x[i * T_LOC:(i + 1) * T_LOC],
            "xt": xt_by_i[i],
            "wt": wt_by_j[j],
            "bb": bias[j * N_LOC:(j + 1) * N_LOC],
            "ws": weight_scale,
            "eye": eye,
        })
    return in_maps


def kernel(x, weight, weight_scale, bias):
    nc = _build()
    in_maps = _stage_inputs(x, weight, weight_scale, bias)
    res = run_bass_kernel_spmd(nc, in_maps, list(range(8))).results

    out = np.empty((TOKENS, OUT_F), dtype=np.float32)
    for c in range(8):
        i, j = c // B_SPLIT, c % B_SPLIT
        out[i * T_LOC:(i + 1) * T_LOC, j * N_LOC:(j + 1) * N_LOC] = res[c]["ob"]
    return out
